# revision 50
# baseline (speedup 1.0000x reference)
"""Trainium2 Bass kernel for nn_DotAttention (B=4, Tq=Tv=2048, D=1024, 16 heads).

Sharding: head-parallel. Core c owns heads (2c, 2c+1) — a 128-wide slice of
the attention dim — and processes ALL 4 batches. Per-batch sequence lengths
(value_lens) become compile-time NJ constants, so short batches cost less on
every core and the load is perfectly balanced (vs. batch-parallel, where the
longest batch's cores dominate).

Pipeline per core (all matmul operands fp16, PSUM f32):
  A: q/k/v projections for the 128-dim head slice, all batches.
  B: attention. energy^T in PSUM [j 128, 2 heads x 512 tq]; exp on ACT with
     per-partition mask bias; ctx via TRANSPOSED matmuls (lhsT = exp tile,
     rhs = v[:, j, 65]) giving [tq 128, 65] at ap=65 — about half the PE
     cycles of the [65, tq] orientation. Column 64 accumulates the softmax
     denominator (ones column in v). Normalize = DVE reciprocal +
     per-partition scalar multiply (no DRAM broadcast bounce), then PE
     transpose back to [att, tq] for the output projection.
  C: y partial = ctxT^T @ Wf[slice] per (batch, tq-tile), fp16 out.

Projection/output work is spliced into the ACT-bound attention loop through
a filler queue so the PE never idles while exps stream. Host sums the 8
per-core partials and adds the constant bv @ Wf + bf.
"""

import sys

sys.path.insert(0, "/opt/trn_rl_repo")

from collections import deque

import numpy as np

import concourse.bacc as bacc
import concourse.tile as tile
import concourse.mybir as mybir
from concourse.bass_utils import run_bass_kernel_spmd

F32 = mybir.dt.float32
F16 = mybir.dt.float16
AF = mybir.ActivationFunctionType

B, T, D, ATT = 4, 2048, 1024, 1024
NH, DH = 16, 64
HPC = 2   # heads per core
CD = 128  # att-dim slice per core
NCORES = 8
LARGE = 1e30
SW = 512  # time-span width per streamed input chunk

# ctx PSUM unit layout: 8 units of 65 f32 cols (2 heads x 4 tq-tiles);
# unit 7 starts at col 512 so no accumulation region crosses a 2KB bank.
UOFF = [0, 65, 130, 195, 260, 325, 390, 512]

_cache = {}


def build_nc(NJS, loop_n=1, popb=1400, ycopy="mix", ebufs=2, exbufs=3,
             chbufs=14, ysbufs=3):
    NJS = tuple(int(x) for x in NJS)
    key = (NJS, loop_n, popb, ycopy, ebufs, exbufs, chbufs, ysbufs)
    if key in _cache:
        return _cache[key]
    NJTOT = sum(NJS)
    KTOT = NJTOT * 128
    joff = [sum(NJS[:s]) for s in range(4)]
    koff = [j * 128 for j in joff]
    NSV = [(nj * 128 + SW - 1) // SW for nj in NJS]

    nc = bacc.Bacc("TRN2", target_bir_lowering=False, debug=False,
                   num_devices=NCORES)

    xq_d = nc.dram_tensor("xq", [4, D, T], F16, kind="ExternalInput")
    xv_d = nc.dram_tensor("xv", [4, D, T], F16, kind="ExternalInput")
    wq_d = nc.dram_tensor("wq", [D, CD], F16, kind="ExternalInput")
    wk_d = nc.dram_tensor("wk", [D, CD], F16, kind="ExternalInput")
    wv_d = nc.dram_tensor("wv", [D, HPC * 65], F16, kind="ExternalInput")
    wf_d = nc.dram_tensor("wf", [CD, ATT], F16, kind="ExternalInput")
    mask_d = nc.dram_tensor("mask", [128, NJTOT], F32, kind="ExternalInput")
    bq_d = nc.dram_tensor("bqc", [128, 1], F32, kind="ExternalInput")
    bk_d = nc.dram_tensor("bkc", [128, 1], F32, kind="ExternalInput")
    id_d = nc.dram_tensor("ident", [128, 128], F16, kind="ExternalInput")
    y_d = nc.dram_tensor("y", [4, T, ATT], F16, kind="ExternalOutput")

    xq_r = xq_d[:, :, :].rearrange("b (kc p) n -> p b kc n", p=128)
    xv_r = xv_d[:, :, :].rearrange("b (kc p) n -> p b kc n", p=128)
    wq_r = wq_d[:, :].rearrange("(kc p) m -> p kc m", p=128)
    wk_r = wk_d[:, :].rearrange("(kc p) m -> p kc m", p=128)
    wv_r = wv_d[:, :].rearrange("(kc p) m -> p kc m", p=128)

    with tile.TileContext(nc) as tc:
        from contextlib import ExitStack
        _st = ExitStack()
        if loop_n > 1:
            _st.enter_context(tc.For_i(0, loop_n, 1))
        with _st, tc.tile_pool(name="persist", bufs=1) as persist, \
                tc.tile_pool(name="chunks", bufs=chbufs) as chunks, \
                tc.tile_pool(name="expp", bufs=exbufs) as expp, \
                tc.tile_pool(name="ctxNp", bufs=4) as ctxNp, \
                tc.tile_pool(name="recp", bufs=4) as recp, \
                tc.tile_pool(name="ysbp", bufs=ysbufs) as ysbp, \
                tc.tile_pool(name="psp", bufs=1, space="PSUM") as psp:
            qT = persist.tile([128, 4, T], F16)
            kT = persist.tile([128, KTOT], F16)
            v = persist.tile([128, NJTOT, HPC * 65], F16)
            ctxT = persist.tile([128, 4, T], F16)
            wqs = persist.tile([128, 8, CD], F16)
            wks = persist.tile([128, 8, CD], F16)
            wvs = persist.tile([128, 8, HPC * 65], F16)
            wfs = persist.tile([128, ATT], F16)
            masks = persist.tile([128, NJTOT], F32)
            bqcs = persist.tile([128, 1], F32)
            bkcs = persist.tile([128, 1], F32)
            ident = persist.tile([128, 128], F16)

            lp = nc.allow_low_precision

            # ---- one-time loads ----
            # order: kv-projection weights first — the lead-in's first
            # matmuls need wks/wvs + the first xv chunk; wq/mask before the
            # xq chunks; wf/ident not until the first boundary
            nc.sync.dma_start(out=wks, in_=wk_r)
            nc.sync.dma_start(out=wvs, in_=wv_r)
            nc.sync.dma_start(out=bkcs, in_=bk_d[:, :])
            nc.sync.dma_start(out=bqcs, in_=bq_d[:, :])

            def mid_loads():
                nc.sync.dma_start(out=masks, in_=mask_d[:, :])
                nc.sync.dma_start(out=wqs, in_=wq_r)

            def late_loads():
                nc.sync.dma_start(out=wfs, in_=wf_d[:, :])
                nc.sync.dma_start(out=ident, in_=id_d[:, :])
            # ones columns of v (written once; v copies skip cols 64/129)
            nc.gpsimd.memset(v[:, :, 64:65], 1.0)
            nc.gpsimd.memset(v[:, :, 129:130], 1.0)

            # ---- chunk DMA + A/C unit helpers ----
            def dma_chunks(s, mid=None):
                """Issue streaming input DMAs for batch-slot s; returns tile
                handles for the A units."""
                xvt = []
                for sp in range(NSV[s]):
                    cnt = min(SW, NJS[s] * 128 - sp * SW)
                    xc = chunks.tile([128, 8, SW], F16, tag="xc", name="xvc")
                    nc.sync.dma_start(
                        out=xc[:, :, 0:cnt],
                        in_=xv_r[:, s, :, sp * SW:sp * SW + cnt])
                    xvt.append((xc, cnt))
                if mid is not None:
                    mid()
                xqt = []
                for sp in range(4):
                    xc = chunks.tile([128, 8, SW], F16, tag="xc", name="xqc")
                    nc.sync.dma_start(out=xc, in_=xq_r[:, s, :, sp * SW:(sp + 1) * SW])
                    xqt.append(xc)
                return xvt, xqt

            # Projection units are split into 4 sub-units of 2 kc-matmuls so
            # the filler granularity matches the per-iteration PE bubble
            # (~1.4k cycles). Sub-units of one unit pop consecutively from
            # the FIFO head, so the shared "small" PSUM slot is never
            # interleaved with another allocation.
            # a_done[key] flips when a projection unit's store has been
            # emitted; need(key) force-pops until then. This is the hard
            # emission-order guarantee that a consumer (energy/ctx) is never
            # emitted before its producer — the Tile dep tracker only orders
            # against already-emitted writers.
            a_done = {}

            def need(key):
                while not a_done.get(key, True):
                    pop_sub()

            def _proj_subs(key, ps_name, mk_mm, mk_store, cnt):
                box = {}
                a_done[key] = False

                def sub(k):
                    def emit():
                        if k == 0:
                            box["ps"] = psp.tile([128, 512], F32, tag="small",
                                                 name=ps_name, bufs=2)
                        for kc in (2 * k, 2 * k + 1):
                            mk_mm(box["ps"], kc)
                        if k == 3:
                            mk_store(box["ps"])
                            a_done[key] = True
                    return (2 * cnt + (600 if k == 3 else 100), emit)

                return [sub(k) for k in range(4)]

            def kt_unit(s, sp, xc, cnt):
                def mm(ps, kc):
                    nc.tensor.matmul(
                        ps[:, 0:cnt], lhsT=wks[:, kc, :],
                        rhs=xc[:, kc, 0:cnt],
                        start=(kc == 0), stop=(kc == 7))

                def store(ps):
                    with lp(reason="kT store"):
                        nc.vector.tensor_scalar_add(
                            kT[:, koff[s] + sp * SW: koff[s] + sp * SW + cnt],
                            ps[:, 0:cnt], bkcs[:, 0:1])
                return _proj_subs(("kt", s, sp), "kps", mm, store, cnt)

            def qt_unit(s, sp, xc):
                def mm(ps, kc):
                    nc.tensor.matmul(
                        ps[:, :], lhsT=wqs[:, kc, :], rhs=xc[:, kc, :],
                        start=(kc == 0), stop=(kc == 7))

                def store(ps):
                    with lp(reason="qT store"):
                        nc.vector.tensor_scalar_add(
                            qT[:, s, sp * SW:(sp + 1) * SW], ps[:, :],
                            bqcs[:, 0:1])
                return _proj_subs(("qt", s, sp), "qps", mm, store, 512)

            def v_unit(s, jl, xc):
                jg = joff[s] + jl
                jt = jl % 4

                def mm(ps, kc):
                    nc.tensor.matmul(
                        ps[:, 0:130], lhsT=xc[:, kc, jt * 128:(jt + 1) * 128],
                        rhs=wvs[:, kc, :], start=(kc == 0), stop=(kc == 7))

                def store(ps):
                    src = ps[:, 0:130].rearrange("p (h x) -> p h x", x=65)
                    dst = v[:, jg, :].rearrange("p (h x) -> p h x", x=65)
                    with lp(reason="v store"):
                        nc.vector.tensor_copy(out=dst[:, :, 0:64],
                                              in_=src[:, :, 0:64])
                return _proj_subs(("v", s, jl), "vps", mm, store, 130)

            def a_units(s, xvt, xqt):
                """Projection units (each a list of sub-units) for slot s,
                kv in j-consumption order."""
                us = []
                for sp in range(NSV[s]):
                    xc, cnt = xvt[sp]
                    us.append(kt_unit(s, sp, xc, cnt))
                    for jl in range(sp * 4, min(sp * 4 + 4, NJS[s])):
                        us.append(v_unit(s, jl, xc))
                for sp in range(4):
                    us.append(qt_unit(s, sp, xqt[sp]))
                return us

            # y accumulates into quarter-slot SBUF tiles ([128, 4, 1024]);
            # one DMA per 4 tq-tiles. Per-unit DMAs would hold the SP
            # sequencer through their waits and serialize everything queued
            # behind them.
            y_sb_tiles = {}
            c_left = {}

            def c_unit(s, i, n):
                def emit():
                    g = i // 4
                    if (s, g) not in y_sb_tiles:
                        y_sb_tiles[(s, g)] = ysbp.tile(
                            [128, 4, ATT], F16, tag="ysb", name=f"ysb{s}_{g}")
                        c_left[(s, g)] = 8
                    yp = psp.tile([128, 512], F32, tag="small", name="yps", bufs=2)
                    nc.tensor.matmul(
                        yp[:, :], lhsT=ctxT[:, s, i * 128:(i + 1) * 128],
                        rhs=wfs[:, n * 512:(n + 1) * 512],
                        start=True, stop=True)
                    # GPSIMD cannot read PSUM on real hw: copies go to DVE
                    # and ACT (Copy activation), alternating to split load
                    dst = y_sb_tiles[(s, g)][:, i % 4, n * 512:(n + 1) * 512]
                    with lp(reason="y store"):
                        if ycopy == "vector" or (ycopy == "mix"
                                                 and (2 * i + n) % 4 != 3):
                            nc.vector.tensor_copy(out=dst, in_=yp[:, :])
                        else:
                            nc.scalar.copy(out=dst, in_=yp[:, :])
                    c_left[(s, g)] -= 1
                    if c_left[(s, g)] == 0:
                        nc.sync.dma_start(
                            out=y_d[s, g * 512:(g + 1) * 512, :].rearrange(
                                "(i p) n -> p i n", p=128),
                            in_=y_sb_tiles.pop((s, g)))
                return (512 + 700, emit)

            def tp_unit(ctx_ps, ctxN, s, ib, g):
                # DMA crossbar transpose: ctxN [128 tq, (h0|h1) 64+64] for
                # tq-tile g -> ctxT[:, s, tile] [128 att, 128 tq]. One DMA
                # per tq-tile; no PE/DVE/ACT engine time at all.
                def emit():
                    nc.sync.dma_start_transpose(
                        out=ctxT[:, s,
                                 ib * 512 + g * 128:ib * 512 + (g + 1) * 128],
                        in_=ctxN[:, g, :, :])
                return (100, emit)

            # ---- main fused schedule ----
            # pending holds (subs, a_slot): subs is a mutable list of
            # (cost, emitfn) sub-units popped one at a time from the head;
            # a_slot >= 0 marks a projection unit for that batch slot.
            pending = deque()
            flat = [(s, ib, j) for s in range(4) for ib in range(4)
                    for j in range(NJS[s])]
            ctx_cur = [None]
            e_tiles = {}

            def pop_sub():
                subs, _ = pending[0]
                cost, fn = subs.pop(0)
                fn()
                if not subs:
                    pending.popleft()
                return cost

            def drain_a_upto(slot):
                # chunk-ring safety: before issuing slot+2's input DMAs,
                # every projection unit reading slot's chunks must be emitted
                while any(u[1] >= 0 and u[1] <= slot for u in pending):
                    pop_sub()

            def emit_energy(idx):
                s, ib, j = flat[idx]
                need(("qt", s, ib))
                need(("kt", s, j // 4))
                if j == 0:
                    ctx_cur[0] = psp.tile([128, 1024], F32, tag="ctx",
                                          name=f"ctx_{s}_{ib}", bufs=1)
                e = psp.tile([128, 1024], F32, tag="e", name="eps", bufs=ebufs)
                for hh in range(2):
                    nc.tensor.matmul(
                        e[:, hh * 512:(hh + 1) * 512],
                        lhsT=kT[hh * 64:(hh + 1) * 64,
                                koff[s] + j * 128:koff[s] + (j + 1) * 128],
                        rhs=qT[hh * 64:(hh + 1) * 64, s,
                               ib * 512:(ib + 1) * 512],
                        start=True, stop=True)
                e_tiles[idx] = (e, ctx_cur[0])

            # lead-in: slot 0 projections (kv + qT span 0 inline; qT spans
            # 1-3 become filler — B(0, ib) only needs qT span ib)
            xvt0, xqt0 = dma_chunks(0, mid=mid_loads)
            late_loads()
            us0 = a_units(0, xvt0, xqt0)
            for unit in us0[:-3]:
                for cost, emitfn in unit:
                    emitfn()
            pending.extend((list(u), 0) for u in us0[-3:])

            # tp/C units of boundary k are emitted/queued at boundary k+1:
            # emitting tp immediately would head-block the PE on the DVE
            # normalize; deferring via the FIFO can invert PE order against
            # the ctx/ctxN ring recycling (deadlock). One-boundary lag is
            # the deterministic middle ground.
            tp_hold, c_hold = [], []

            def flush_boundary():
                for u in tp_hold:
                    u()
                tp_hold.clear()
                pending.extend(([cf], -1) for cf in c_hold)
                c_hold.clear()

            # normalize of boundary k is deferred to iteration k+1 (just
            # before the next block's first ctx write — the latest legal
            # point for the ctx ring) so the DVE never camps on its SEQ
            # waiting for the closing ctx matmuls.
            norm_hold = []

            def norm_unit(ctx_ps, s, ib):
                def emit():
                    rec = recp.tile([128, 8], F32, tag="rec", name="rec")
                    # batched reciprocal: units 0-6 denominators sit at a
                    # uniform 65-column pitch; unit 7 is the bank-1 outlier
                    dstr = ctx_ps[:, 0:455].rearrange("p (u x) -> p u x",
                                                      x=65)
                    nc.vector.reciprocal(rec[:, 0:7], dstr[:, :, 64])
                    nc.vector.reciprocal(rec[:, 7:8], ctx_ps[:, 576:577])
                    # ctxN units keyed (tq-tile, head): each tq-tile's two
                    # heads are contiguous 128 cols for the xbar transpose
                    ctxN = ctxNp.tile([128, 4, 2, 64], F16, tag="cn",
                                      name="ctxN")
                    for u in range(8):
                        hh, tt = u // 4, u % 4
                        with lp(reason="ctx normalize"):
                            nc.vector.tensor_scalar_mul(
                                ctxN[:, tt, hh, :],
                                ctx_ps[:, UOFF[u]:UOFF[u] + 64],
                                rec[:, u:u + 1])
                    # transposes go out immediately: the SP queue absorbs
                    # the wait-for-muls, giving C units a full boundary of
                    # DMA latency slack before they pop
                    for g in range(4):
                        tp_unit(ctx_ps, ctxN, s, ib, g)[1]()
                    c_hold.extend(
                        c_unit(s, i, n)
                        for i in range(ib * 4, ib * 4 + 4) for n in range(2))
                return emit

            emit_energy(0)
            budget = 0.0
            for idx, (s, ib, j) in enumerate(flat):
                if ib == 0 and j == 0 and s + 1 < 4:
                    # chunk-ring (bufs=15) reuse reaches two slots back, so
                    # only slot s-1's units must be flushed; issuing the
                    # chunk DMAs FIRST keeps them ahead of y DMAs on SP
                    drain_a_upto(s - 1)
                    xvt, xqt = dma_chunks(s + 1)
                    pending.extend((list(u), s + 1)
                                   for u in a_units(s + 1, xvt, xqt))
                e, ctx_ps = e_tiles.pop(idx)
                ex = expp.tile([128, 1024], F16, tag="ex", name="ex")
                with lp(reason="exp fp16"):
                    nc.scalar.activation(
                        out=ex[:, :], in_=e[:, :], func=AF.Exp,
                        bias=masks[:, joff[s] + j:joff[s] + j + 1], scale=1.0)
                if idx + 1 < len(flat):
                    emit_energy(idx + 1)
                for u in norm_hold:
                    u()
                norm_hold.clear()
                # filler BETWEEN energy(idx+1) and ctx(idx): the in-order PE
                # queue blocks at ctx(idx) until exp(idx) lands, so filler
                # emitted after it would not run during that window
                budget += popb * (2 if s == 3 else 1)
                while pending and budget >= pending[0][0][0][0]:
                    budget -= pop_sub()
                need(("v", s, j))
                for u in range(8):
                    hh, tt = u // 4, u % 4
                    # one accumulation group per 2KB zero region (bank):
                    # units 0-6 live in bank 0, unit 7 in bank 1. start on
                    # the region's first matmul pending-zeroes the whole
                    # region; units 1-6 at j==0 overwrite via pending-zero.
                    nc.tensor.matmul(
                        ctx_ps[:, UOFF[u]:UOFF[u] + 65],
                        lhsT=ex[:, hh * 512 + tt * 128:hh * 512 + (tt + 1) * 128],
                        rhs=v[:, joff[s] + j, hh * 65:(hh + 1) * 65],
                        start=(j == 0 and u in (0, 7)),
                        stop=(j == NJS[s] - 1 and u in (6, 7)))
                if j == NJS[s] - 1:
                    flush_boundary()
                    norm_hold.append(norm_unit(ctx_ps, s, ib))
            for u in norm_hold:
                u()
            norm_hold.clear()
            flush_boundary()
            while pending:
                pop_sub()
    nc.compile()
    _cache[key] = nc
    return nc


def make_in_maps(query, value, value_lens, Wq, bq, Wk, bk, Wv, bv, Wf, bf):
    query = np.asarray(query, np.float32)
    value = np.asarray(value, np.float32)
    value_lens = np.asarray(value_lens)
    Wq = np.asarray(Wq, np.float32)
    Wk = np.asarray(Wk, np.float32)
    Wv = np.asarray(Wv, np.float32)
    Wf = np.asarray(Wf, np.float32)
    bq = np.asarray(bq, np.float32)
    bk = np.asarray(bk, np.float32)

    scale = 1.0 / np.sqrt(np.float32(DH))
    effL = [int(l) if l > 0 else T for l in value_lens]
    order = sorted(range(B), key=lambda b: effL[b])  # slot -> batch
    NJS = tuple(max(1, int(np.ceil(effL[b] / 128))) for b in order)
    NJTOT = sum(NJS)
    joff = [sum(NJS[:s]) for s in range(4)]

    # shared across cores
    xq = np.ascontiguousarray(
        query.transpose(0, 2, 1)[order].astype(np.float16))
    xv = np.ascontiguousarray(
        value.transpose(0, 2, 1)[order].astype(np.float16))
    mask = np.zeros((128, NJTOT), np.float32)
    for s in range(4):
        L = effL[order[s]]
        idx = np.arange(NJS[s] * 128).reshape(NJS[s], 128).T  # [128, NJ_s]
        m = np.zeros((128, NJS[s]), np.float32)
        m[idx >= L] = -LARGE
        mask[:, joff[s]:joff[s] + NJS[s]] = m
    ident = np.eye(128, dtype=np.float16)

    in_maps = []
    for c in range(NCORES):
        cs = slice(c * CD, (c + 1) * CD)
        wq = (Wq[:, cs] * scale).astype(np.float16)
        wk = Wk[:, cs].astype(np.float16)
        wv = np.zeros((D, HPC * 65), np.float16)
        for h in range(HPC):
            wv[:, h * 65:h * 65 + 64] = Wv[:, c * CD + h * 64:c * CD + (h + 1) * 64]
        wf = Wf[cs, :].astype(np.float16)
        bqc = (bq[cs] * scale).reshape(CD, 1).astype(np.float32)
        bkc = bk[cs].reshape(CD, 1).astype(np.float32)
        in_maps.append({
            "xq": xq, "xv": xv, "wq": wq, "wk": wk, "wv": wv, "wf": wf,
            "mask": mask, "bqc": bqc, "bkc": bkc, "ident": ident,
        })
    return in_maps, NJS


def assemble(results, value_lens, Wv, bv, Wf, bf):
    value_lens = np.asarray(value_lens)
    bv = np.asarray(bv, np.float32)
    Wf = np.asarray(Wf, np.float32)
    bf = np.asarray(bf, np.float32)
    effL = [int(l) if l > 0 else T for l in value_lens]
    order = sorted(range(B), key=lambda b: effL[b])
    acc = results[0]["y"].astype(np.float32)
    for c in range(1, NCORES):
        acc += results[c]["y"]
    const = (bv @ Wf + bf).astype(np.float32)
    out = np.empty((B, T, ATT), np.float32)
    for s in range(4):
        out[order[s]] = acc[s] + const
    return out


def kernel(query, value, value_lens, Wq, bq, Wk, bk, Wv, bv, Wf, bf):
    in_maps, NJS = make_in_maps(query, value, value_lens, Wq, bq, Wk, bk,
                                Wv, bv, Wf, bf)
    nc = build_nc(NJS)
    res = run_bass_kernel_spmd(nc, in_maps, list(range(NCORES)))
    return assemble(res.results, value_lens, Wv, bv, Wf, bf)


# revision 52
# speedup vs baseline: 1.0037x; 1.0037x over previous
"""Trainium2 Bass kernel for nn_DotAttention (B=4, Tq=Tv=2048, D=1024, 16 heads).

Sharding: head-parallel. Core c owns heads (2c, 2c+1) — a 128-wide slice of
the attention dim — and processes ALL 4 batches. Per-batch sequence lengths
(value_lens) become compile-time NJ constants, so short batches cost less on
every core and the load is perfectly balanced (vs. batch-parallel, where the
longest batch's cores dominate).

Pipeline per core (all matmul operands fp16, PSUM f32):
  A: q/k/v projections for the 128-dim head slice, all batches.
  B: attention. energy^T in PSUM [j 128, 2 heads x 512 tq]; exp on ACT with
     per-partition mask bias; ctx via TRANSPOSED matmuls (lhsT = exp tile,
     rhs = v[:, j, 65]) giving [tq 128, 65] at ap=65 — about half the PE
     cycles of the [65, tq] orientation. Column 64 accumulates the softmax
     denominator (ones column in v). Normalize = DVE reciprocal +
     per-partition scalar multiply (no DRAM broadcast bounce), then a DMA
     crossbar transpose back to [att, tq] for the output projection.
  C: y partial = ctxT^T @ Wf[slice] per (batch, tq-tile), fp16 out.

Projection/output work is spliced into the ACT-bound attention loop through
a filler queue so the PE never idles while exps stream. Host sums the 8
per-core partials and adds the constant bv @ Wf + bf.
"""

import sys

sys.path.insert(0, "/opt/trn_rl_repo")

from collections import deque

import numpy as np

import concourse.bacc as bacc
import concourse.tile as tile
import concourse.mybir as mybir
from concourse.bass_utils import run_bass_kernel_spmd

F32 = mybir.dt.float32
F16 = mybir.dt.float16
AF = mybir.ActivationFunctionType

B, T, D, ATT = 4, 2048, 1024, 1024
NH, DH = 16, 64
HPC = 2   # heads per core
CD = 128  # att-dim slice per core
NCORES = 8
LARGE = 1e30
SW = 512  # time-span width per streamed input chunk

# ctx PSUM unit layout: 8 units of 65 f32 cols (2 heads x 4 tq-tiles);
# unit 7 starts at col 512 so no accumulation region crosses a 2KB bank.
UOFF = [0, 65, 130, 195, 260, 325, 390, 512]

_cache = {}


def build_nc(NJS, loop_n=1, popb=1400, ycopy="mix", ebufs=2, exbufs=3,
             chbufs=14, ysbufs=3, ymod=2, s3boost=2):
    NJS = tuple(int(x) for x in NJS)
    key = (NJS, loop_n, popb, ycopy, ebufs, exbufs, chbufs, ysbufs, ymod, s3boost)
    if key in _cache:
        return _cache[key]
    NJTOT = sum(NJS)
    KTOT = NJTOT * 128
    joff = [sum(NJS[:s]) for s in range(4)]
    koff = [j * 128 for j in joff]
    NSV = [(nj * 128 + SW - 1) // SW for nj in NJS]

    nc = bacc.Bacc("TRN2", target_bir_lowering=False, debug=False,
                   num_devices=NCORES)

    xq_d = nc.dram_tensor("xq", [4, D, T], F16, kind="ExternalInput")
    xv_d = nc.dram_tensor("xv", [4, D, T], F16, kind="ExternalInput")
    wq_d = nc.dram_tensor("wq", [D, CD], F16, kind="ExternalInput")
    wk_d = nc.dram_tensor("wk", [D, CD], F16, kind="ExternalInput")
    wv_d = nc.dram_tensor("wv", [D, HPC * 65], F16, kind="ExternalInput")
    wf_d = nc.dram_tensor("wf", [CD, ATT], F16, kind="ExternalInput")
    mask_d = nc.dram_tensor("mask", [128, NJTOT], F32, kind="ExternalInput")
    bq_d = nc.dram_tensor("bqc", [128, 1], F32, kind="ExternalInput")
    bk_d = nc.dram_tensor("bkc", [128, 1], F32, kind="ExternalInput")
    id_d = nc.dram_tensor("ident", [128, 128], F16, kind="ExternalInput")
    y_d = nc.dram_tensor("y", [4, T, ATT], F16, kind="ExternalOutput")

    xq_r = xq_d[:, :, :].rearrange("b (kc p) n -> p b kc n", p=128)
    xv_r = xv_d[:, :, :].rearrange("b (kc p) n -> p b kc n", p=128)
    wq_r = wq_d[:, :].rearrange("(kc p) m -> p kc m", p=128)
    wk_r = wk_d[:, :].rearrange("(kc p) m -> p kc m", p=128)
    wv_r = wv_d[:, :].rearrange("(kc p) m -> p kc m", p=128)

    with tile.TileContext(nc) as tc:
        from contextlib import ExitStack
        _st = ExitStack()
        if loop_n > 1:
            _st.enter_context(tc.For_i(0, loop_n, 1))
        with _st, tc.tile_pool(name="persist", bufs=1) as persist, \
                tc.tile_pool(name="chunks", bufs=chbufs) as chunks, \
                tc.tile_pool(name="expp", bufs=exbufs) as expp, \
                tc.tile_pool(name="ctxNp", bufs=4) as ctxNp, \
                tc.tile_pool(name="recp", bufs=4) as recp, \
                tc.tile_pool(name="ysbp", bufs=ysbufs) as ysbp, \
                tc.tile_pool(name="psp", bufs=1, space="PSUM") as psp:
            qT = persist.tile([128, 4, T], F16)
            kT = persist.tile([128, KTOT], F16)
            v = persist.tile([128, NJTOT, HPC * 65], F16)
            ctxT = persist.tile([128, 4, T], F16)
            wqs = persist.tile([128, 8, CD], F16)
            wks = persist.tile([128, 8, CD], F16)
            wvs = persist.tile([128, 8, HPC * 65], F16)
            wfs = persist.tile([128, ATT], F16)
            masks = persist.tile([128, NJTOT], F32)
            bqcs = persist.tile([128, 1], F32)
            bkcs = persist.tile([128, 1], F32)
            ident = persist.tile([128, 128], F16)

            lp = nc.allow_low_precision

            # ---- one-time loads ----
            # order: kv-projection weights first — the lead-in's first
            # matmuls need wks/wvs + the first xv chunk; wq/mask before the
            # xq chunks; wf/ident not until the first boundary
            nc.sync.dma_start(out=wks, in_=wk_r)
            nc.sync.dma_start(out=wvs, in_=wv_r)
            nc.sync.dma_start(out=bkcs, in_=bk_d[:, :])
            nc.sync.dma_start(out=bqcs, in_=bq_d[:, :])

            def mid_loads():
                nc.sync.dma_start(out=masks, in_=mask_d[:, :])
                nc.sync.dma_start(out=wqs, in_=wq_r)

            def late_loads():
                nc.sync.dma_start(out=wfs, in_=wf_d[:, :])
                nc.sync.dma_start(out=ident, in_=id_d[:, :])
            # ones columns of v (written once; v copies skip cols 64/129)
            nc.gpsimd.memset(v[:, :, 64:65], 1.0)
            nc.gpsimd.memset(v[:, :, 129:130], 1.0)

            # ---- chunk DMA + A/C unit helpers ----
            def dma_chunks(s, mid=None):
                """Issue streaming input DMAs for batch-slot s; returns tile
                handles for the A units."""
                xvt = []
                for sp in range(NSV[s]):
                    cnt = min(SW, NJS[s] * 128 - sp * SW)
                    xc = chunks.tile([128, 8, SW], F16, tag="xc", name="xvc")
                    nc.sync.dma_start(
                        out=xc[:, :, 0:cnt],
                        in_=xv_r[:, s, :, sp * SW:sp * SW + cnt])
                    xvt.append((xc, cnt))
                if mid is not None:
                    mid()
                xqt = []
                for sp in range(4):
                    xc = chunks.tile([128, 8, SW], F16, tag="xc", name="xqc")
                    nc.sync.dma_start(out=xc, in_=xq_r[:, s, :, sp * SW:(sp + 1) * SW])
                    xqt.append(xc)
                return xvt, xqt

            # Projection units are split into 4 sub-units of 2 kc-matmuls so
            # the filler granularity matches the per-iteration PE bubble
            # (~1.4k cycles). Sub-units of one unit pop consecutively from
            # the FIFO head, so the shared "small" PSUM slot is never
            # interleaved with another allocation.
            # a_done[key] flips when a projection unit's store has been
            # emitted; need(key) force-pops until then. This is the hard
            # emission-order guarantee that a consumer (energy/ctx) is never
            # emitted before its producer — the Tile dep tracker only orders
            # against already-emitted writers.
            a_done = {}

            def need(key):
                while not a_done.get(key, True):
                    pop_sub()

            def _proj_subs(key, ps_name, mk_mm, mk_store, cnt):
                box = {}
                a_done[key] = False

                def sub(k):
                    def emit():
                        if k == 0:
                            box["ps"] = psp.tile([128, 512], F32, tag="small",
                                                 name=ps_name, bufs=2)
                        for kc in (2 * k, 2 * k + 1):
                            mk_mm(box["ps"], kc)
                        if k == 3:
                            mk_store(box["ps"])
                            a_done[key] = True
                    return (2 * cnt + (600 if k == 3 else 100), emit)

                return [sub(k) for k in range(4)]

            def kt_unit(s, sp, xc, cnt):
                def mm(ps, kc):
                    nc.tensor.matmul(
                        ps[:, 0:cnt], lhsT=wks[:, kc, :],
                        rhs=xc[:, kc, 0:cnt],
                        start=(kc == 0), stop=(kc == 7))

                def store(ps):
                    with lp(reason="kT store"):
                        nc.vector.tensor_scalar_add(
                            kT[:, koff[s] + sp * SW: koff[s] + sp * SW + cnt],
                            ps[:, 0:cnt], bkcs[:, 0:1])
                return _proj_subs(("kt", s, sp), "kps", mm, store, cnt)

            def qt_unit(s, sp, xc):
                def mm(ps, kc):
                    nc.tensor.matmul(
                        ps[:, :], lhsT=wqs[:, kc, :], rhs=xc[:, kc, :],
                        start=(kc == 0), stop=(kc == 7))

                def store(ps):
                    with lp(reason="qT store"):
                        nc.vector.tensor_scalar_add(
                            qT[:, s, sp * SW:(sp + 1) * SW], ps[:, :],
                            bqcs[:, 0:1])
                return _proj_subs(("qt", s, sp), "qps", mm, store, 512)

            def v_unit(s, jl, xc):
                jg = joff[s] + jl
                jt = jl % 4

                def mm(ps, kc):
                    nc.tensor.matmul(
                        ps[:, 0:130], lhsT=xc[:, kc, jt * 128:(jt + 1) * 128],
                        rhs=wvs[:, kc, :], start=(kc == 0), stop=(kc == 7))

                def store(ps):
                    src = ps[:, 0:130].rearrange("p (h x) -> p h x", x=65)
                    dst = v[:, jg, :].rearrange("p (h x) -> p h x", x=65)
                    with lp(reason="v store"):
                        nc.vector.tensor_copy(out=dst[:, :, 0:64],
                                              in_=src[:, :, 0:64])
                return _proj_subs(("v", s, jl), "vps", mm, store, 130)

            def a_units(s, xvt, xqt):
                """Projection units (each a list of sub-units) for slot s,
                kv in j-consumption order."""
                us = []
                for sp in range(NSV[s]):
                    xc, cnt = xvt[sp]
                    us.append(kt_unit(s, sp, xc, cnt))
                    for jl in range(sp * 4, min(sp * 4 + 4, NJS[s])):
                        us.append(v_unit(s, jl, xc))
                for sp in range(4):
                    us.append(qt_unit(s, sp, xqt[sp]))
                return us

            # y accumulates into quarter-slot SBUF tiles ([128, 4, 1024]);
            # one DMA per 4 tq-tiles. Per-unit DMAs would hold the SP
            # sequencer through their waits and serialize everything queued
            # behind them.
            y_sb_tiles = {}
            c_left = {}

            def c_unit(s, i, n):
                def emit():
                    g = i // 4
                    if (s, g) not in y_sb_tiles:
                        y_sb_tiles[(s, g)] = ysbp.tile(
                            [128, 4, ATT], F16, tag="ysb", name=f"ysb{s}_{g}")
                        c_left[(s, g)] = 8
                    yp = psp.tile([128, 512], F32, tag="small", name="yps", bufs=2)
                    nc.tensor.matmul(
                        yp[:, :], lhsT=ctxT[:, s, i * 128:(i + 1) * 128],
                        rhs=wfs[:, n * 512:(n + 1) * 512],
                        start=True, stop=True)
                    # GPSIMD cannot read PSUM on real hw: copies go to DVE
                    # and ACT (Copy activation), alternating to split load
                    dst = y_sb_tiles[(s, g)][:, i % 4, n * 512:(n + 1) * 512]
                    with lp(reason="y store"):
                        if ycopy == "vector" or (ycopy == "mix"
                                                 and (2 * i + n) % ymod != ymod - 1):
                            nc.vector.tensor_copy(out=dst, in_=yp[:, :])
                        else:
                            nc.scalar.copy(out=dst, in_=yp[:, :])
                    c_left[(s, g)] -= 1
                    if c_left[(s, g)] == 0:
                        nc.sync.dma_start(
                            out=y_d[s, g * 512:(g + 1) * 512, :].rearrange(
                                "(i p) n -> p i n", p=128),
                            in_=y_sb_tiles.pop((s, g)))
                return (512 + 700, emit)

            def tp_unit(ctx_ps, ctxN, s, ib, g):
                # DMA crossbar transpose: ctxN [128 tq, (h0|h1) 64+64] for
                # tq-tile g -> ctxT[:, s, tile] [128 att, 128 tq]. One DMA
                # per tq-tile; no PE/DVE/ACT engine time at all.
                def emit():
                    nc.sync.dma_start_transpose(
                        out=ctxT[:, s,
                                 ib * 512 + g * 128:ib * 512 + (g + 1) * 128],
                        in_=ctxN[:, g, :, :])
                return (100, emit)

            # ---- main fused schedule ----
            # pending holds (subs, a_slot): subs is a mutable list of
            # (cost, emitfn) sub-units popped one at a time from the head;
            # a_slot >= 0 marks a projection unit for that batch slot.
            pending = deque()
            flat = [(s, ib, j) for s in range(4) for ib in range(4)
                    for j in range(NJS[s])]
            ctx_cur = [None]
            e_tiles = {}

            def pop_sub():
                subs, _ = pending[0]
                cost, fn = subs.pop(0)
                fn()
                if not subs:
                    pending.popleft()
                return cost

            def drain_a_upto(slot):
                # chunk-ring safety: before issuing slot+2's input DMAs,
                # every projection unit reading slot's chunks must be emitted
                while any(u[1] >= 0 and u[1] <= slot for u in pending):
                    pop_sub()

            def emit_energy(idx):
                s, ib, j = flat[idx]
                need(("qt", s, ib))
                need(("kt", s, j // 4))
                if j == 0:
                    ctx_cur[0] = psp.tile([128, 1024], F32, tag="ctx",
                                          name=f"ctx_{s}_{ib}", bufs=1)
                e = psp.tile([128, 1024], F32, tag="e", name="eps", bufs=ebufs)
                for hh in range(2):
                    nc.tensor.matmul(
                        e[:, hh * 512:(hh + 1) * 512],
                        lhsT=kT[hh * 64:(hh + 1) * 64,
                                koff[s] + j * 128:koff[s] + (j + 1) * 128],
                        rhs=qT[hh * 64:(hh + 1) * 64, s,
                               ib * 512:(ib + 1) * 512],
                        start=True, stop=True)
                e_tiles[idx] = (e, ctx_cur[0])

            # lead-in: slot 0 projections (kv + qT span 0 inline; qT spans
            # 1-3 become filler — B(0, ib) only needs qT span ib)
            xvt0, xqt0 = dma_chunks(0, mid=mid_loads)
            late_loads()
            us0 = a_units(0, xvt0, xqt0)
            for unit in us0[:-3]:
                for cost, emitfn in unit:
                    emitfn()
            pending.extend((list(u), 0) for u in us0[-3:])

            # tp/C units of boundary k are emitted/queued at boundary k+1:
            # emitting tp immediately would head-block the PE on the DVE
            # normalize; deferring via the FIFO can invert PE order against
            # the ctx/ctxN ring recycling (deadlock). One-boundary lag is
            # the deterministic middle ground.
            tp_hold, c_hold = [], []

            def flush_boundary():
                for u in tp_hold:
                    u()
                tp_hold.clear()
                pending.extend(([cf], -1) for cf in c_hold)
                c_hold.clear()

            # normalize of boundary k is deferred to iteration k+1 (just
            # before the next block's first ctx write — the latest legal
            # point for the ctx ring) so the DVE never camps on its SEQ
            # waiting for the closing ctx matmuls.
            norm_hold = []

            def norm_unit(ctx_ps, s, ib):
                def emit():
                    rec = recp.tile([128, 8], F32, tag="rec", name="rec")
                    # batched reciprocal: units 0-6 denominators sit at a
                    # uniform 65-column pitch; unit 7 is the bank-1 outlier
                    dstr = ctx_ps[:, 0:455].rearrange("p (u x) -> p u x",
                                                      x=65)
                    nc.vector.reciprocal(rec[:, 0:7], dstr[:, :, 64])
                    nc.vector.reciprocal(rec[:, 7:8], ctx_ps[:, 576:577])
                    # ctxN units keyed (tq-tile, head): each tq-tile's two
                    # heads are contiguous 128 cols for the xbar transpose
                    ctxN = ctxNp.tile([128, 4, 2, 64], F16, tag="cn",
                                      name="ctxN")
                    for u in range(8):
                        hh, tt = u // 4, u % 4
                        with lp(reason="ctx normalize"):
                            nc.vector.tensor_scalar_mul(
                                ctxN[:, tt, hh, :],
                                ctx_ps[:, UOFF[u]:UOFF[u] + 64],
                                rec[:, u:u + 1])
                    # transposes go out immediately: the SP queue absorbs
                    # the wait-for-muls, giving C units a full boundary of
                    # DMA latency slack before they pop
                    for g in range(4):
                        tp_unit(ctx_ps, ctxN, s, ib, g)[1]()
                    c_hold.extend(
                        c_unit(s, i, n)
                        for i in range(ib * 4, ib * 4 + 4) for n in range(2))
                return emit

            emit_energy(0)
            budget = 0.0
            for idx, (s, ib, j) in enumerate(flat):
                if ib == 0 and j == 0 and s + 1 < 4:
                    # chunk-ring (bufs=15) reuse reaches two slots back, so
                    # only slot s-1's units must be flushed; issuing the
                    # chunk DMAs FIRST keeps them ahead of y DMAs on SP
                    drain_a_upto(s - 1)
                    xvt, xqt = dma_chunks(s + 1)
                    pending.extend((list(u), s + 1)
                                   for u in a_units(s + 1, xvt, xqt))
                e, ctx_ps = e_tiles.pop(idx)
                ex = expp.tile([128, 1024], F16, tag="ex", name="ex")
                with lp(reason="exp fp16"):
                    nc.scalar.activation(
                        out=ex[:, :], in_=e[:, :], func=AF.Exp,
                        bias=masks[:, joff[s] + j:joff[s] + j + 1], scale=1.0)
                if idx + 1 < len(flat):
                    emit_energy(idx + 1)
                for u in norm_hold:
                    u()
                norm_hold.clear()
                # filler BETWEEN energy(idx+1) and ctx(idx): the in-order PE
                # queue blocks at ctx(idx) until exp(idx) lands, so filler
                # emitted after it would not run during that window
                budget += popb * (s3boost if s == 3 else 1)
                while pending and budget >= pending[0][0][0][0]:
                    budget -= pop_sub()
                need(("v", s, j))
                for u in range(8):
                    hh, tt = u // 4, u % 4
                    # one accumulation group per 2KB zero region (bank):
                    # units 0-6 live in bank 0, unit 7 in bank 1. start on
                    # the region's first matmul pending-zeroes the whole
                    # region; units 1-6 at j==0 overwrite via pending-zero.
                    nc.tensor.matmul(
                        ctx_ps[:, UOFF[u]:UOFF[u] + 65],
                        lhsT=ex[:, hh * 512 + tt * 128:hh * 512 + (tt + 1) * 128],
                        rhs=v[:, joff[s] + j, hh * 65:(hh + 1) * 65],
                        start=(j == 0 and u in (0, 7)),
                        stop=(j == NJS[s] - 1 and u in (6, 7)))
                if j == NJS[s] - 1:
                    flush_boundary()
                    norm_hold.append(norm_unit(ctx_ps, s, ib))
            for u in norm_hold:
                u()
            norm_hold.clear()
            flush_boundary()
            while pending:
                pop_sub()
    nc.compile()
    _cache[key] = nc
    return nc


def make_in_maps(query, value, value_lens, Wq, bq, Wk, bk, Wv, bv, Wf, bf):
    query = np.asarray(query, np.float32)
    value = np.asarray(value, np.float32)
    value_lens = np.asarray(value_lens)
    Wq = np.asarray(Wq, np.float32)
    Wk = np.asarray(Wk, np.float32)
    Wv = np.asarray(Wv, np.float32)
    Wf = np.asarray(Wf, np.float32)
    bq = np.asarray(bq, np.float32)
    bk = np.asarray(bk, np.float32)

    scale = 1.0 / np.sqrt(np.float32(DH))
    effL = [int(l) if l > 0 else T for l in value_lens]
    order = sorted(range(B), key=lambda b: effL[b])  # slot -> batch
    NJS = tuple(max(1, int(np.ceil(effL[b] / 128))) for b in order)
    NJTOT = sum(NJS)
    joff = [sum(NJS[:s]) for s in range(4)]

    # shared across cores
    xq = np.ascontiguousarray(
        query.transpose(0, 2, 1)[order].astype(np.float16))
    xv = np.ascontiguousarray(
        value.transpose(0, 2, 1)[order].astype(np.float16))
    mask = np.zeros((128, NJTOT), np.float32)
    for s in range(4):
        L = effL[order[s]]
        idx = np.arange(NJS[s] * 128).reshape(NJS[s], 128).T  # [128, NJ_s]
        m = np.zeros((128, NJS[s]), np.float32)
        m[idx >= L] = -LARGE
        mask[:, joff[s]:joff[s] + NJS[s]] = m
    ident = np.eye(128, dtype=np.float16)

    in_maps = []
    for c in range(NCORES):
        cs = slice(c * CD, (c + 1) * CD)
        wq = (Wq[:, cs] * scale).astype(np.float16)
        wk = Wk[:, cs].astype(np.float16)
        wv = np.zeros((D, HPC * 65), np.float16)
        for h in range(HPC):
            wv[:, h * 65:h * 65 + 64] = Wv[:, c * CD + h * 64:c * CD + (h + 1) * 64]
        wf = Wf[cs, :].astype(np.float16)
        bqc = (bq[cs] * scale).reshape(CD, 1).astype(np.float32)
        bkc = bk[cs].reshape(CD, 1).astype(np.float32)
        in_maps.append({
            "xq": xq, "xv": xv, "wq": wq, "wk": wk, "wv": wv, "wf": wf,
            "mask": mask, "bqc": bqc, "bkc": bkc, "ident": ident,
        })
    return in_maps, NJS


def assemble(results, value_lens, Wv, bv, Wf, bf):
    value_lens = np.asarray(value_lens)
    bv = np.asarray(bv, np.float32)
    Wf = np.asarray(Wf, np.float32)
    bf = np.asarray(bf, np.float32)
    effL = [int(l) if l > 0 else T for l in value_lens]
    order = sorted(range(B), key=lambda b: effL[b])
    acc = results[0]["y"].astype(np.float32)
    for c in range(1, NCORES):
        acc += results[c]["y"]
    const = (bv @ Wf + bf).astype(np.float32)
    out = np.empty((B, T, ATT), np.float32)
    for s in range(4):
        out[order[s]] = acc[s] + const
    return out


def kernel(query, value, value_lens, Wq, bq, Wk, bk, Wv, bv, Wf, bf):
    in_maps, NJS = make_in_maps(query, value, value_lens, Wq, bq, Wk, bk,
                                Wv, bv, Wf, bf)
    nc = build_nc(NJS)
    res = run_bass_kernel_spmd(nc, in_maps, list(range(NCORES)))
    return assemble(res.results, value_lens, Wv, bv, Wf, bf)
